# revision 1
# baseline (speedup 1.0000x reference)
"""GCN (3-layer + readout) on 8 Trainium2 NeuronCores.

Strategy (dst-node sharding, 1D graph parallel):
  - Nodes are sharded across 8 cores (6250/core, padded to 6272 = 49 blocks
    of 128).  Each core aggregates messages for the edges whose dst lands in
    its shard.
  - Per layer: transform z = h @ W (PE), scale rows by dinv = deg^-1/2 so
    table rows are dinv[src]*z[src].  Layer 1's table is computed fully
    locally by every core (x is replicated; each core gets a node
    permutation with its own shard first so the self-term slice is uniform
    across the SPMD program).  Layers 2/3 AllGather the shard tables.
  - Edge gathers: one indirect DMA (standard InstDMACopy dynamic-offset
    SWDGE path, int32 row ids, one row per partition) per 128-edge chunk.
  - Scatter-add on the TensorEngine: per chunk a one-hot
    onehot[e, d] = (dst_local[e] == d) is built with one DVE
    tensor_scalar(is_equal) against an iota row; psum[128d, 64] +=
    onehot^T @ msg accumulates the dst-block aggregation.
  - Self loops (PyG GCNConv implicit): the own-shard table slice is kept in
    SBUF and added to the block aggregate before the dst-side dinv scale,
    yielding exactly dinv^2 * z.
  - Host-side preprocessing is strictly index/metadata work (edge bucketing,
    padding, degree counting, node permutation); all float math runs on
    device.
"""

import numpy as np

from concourse import bacc, bass, mybir, tile
from concourse.bass_utils import run_bass_kernel_spmd

# ---------------------------------------------------------------- constants
P = 8                      # cores
N = 50000                  # nodes
IN_DIM = 128
HID = 64
OUT_DIM = 10
BLK = 128

F32 = mybir.dt.float32
I32 = mybir.dt.int32


def _derived():
    shard = N // P
    nblk = (shard + BLK - 1) // BLK
    pads = nblk * BLK
    tbl = P * pads
    return shard, nblk, pads, tbl


def _refresh_dims():
    global SHARD, NBLK, PADS, TBL
    SHARD, NBLK, PADS, TBL = _derived()


SHARD, NBLK, PADS, TBL = _derived()


# ------------------------------------------------------------- host prep
def _preprocess(x, edge_index):
    """Bucket edges into per-(core, dst-block) 128-edge chunks.

    Nodes are bin-packed into the P*NBLK (core, block) bins by in-degree
    (capacity-constrained LPT) so every bin carries ~the same edge count —
    this minimizes the uniform per-block chunk counts, which set the Q7
    gather-instruction floor.
    """
    import heapq

    x = np.asarray(x, np.float32)
    ei = np.asarray(edge_index, np.int64)
    src, dst = ei[0], ei[1]

    degE = np.bincount(dst, minlength=N).astype(np.int64)
    deg = (degE + 1).astype(np.float32)

    NBINS = P * NBLK
    order_n = np.argsort(-degE, kind="stable")
    heap = [(0, b) for b in range(NBINS)]
    heapq.heapify(heap)
    fill = np.zeros(NBINS, np.int64)
    node_bin = np.empty(N, np.int64)
    node_slot = np.empty(N, np.int64)
    for n in order_n:
        while True:
            s, b = heapq.heappop(heap)
            if fill[b] < BLK:
                break
        node_bin[n] = b
        node_slot[n] = fill[b]
        fill[b] += 1
        heapq.heappush(heap, (s + int(degE[n]), b))

    newid = node_bin * BLK + node_slot          # padded global row of each node

    owner = node_bin[dst] // NBLK
    blk = node_bin[dst] % NBLK
    dstl = node_slot[dst].astype(np.float32)
    s_own = node_bin[src] // NBLK
    s_loc = (node_bin[src] % NBLK) * BLK + node_slot[src]   # padded local row

    # chunk slots
    gid = owner * NBLK + blk
    order = np.argsort(gid, kind="stable")
    gid_s = gid[order]
    counts = np.bincount(gid_s, minlength=P * NBLK)
    starts = np.concatenate([[0], np.cumsum(counts)[:-1]])
    pos = np.arange(gid_s.size) - starts[gid_s]

    # per-block chunk count: max over cores (program must be core-uniform)
    C_arr = np.maximum(
        np.ceil(counts.reshape(P, NBLK).max(axis=0) / BLK).astype(np.int64), 1)
    base = np.concatenate([[0], np.cumsum(C_arr)[:-1]])
    T = int(C_arr.sum())

    own_s = gid_s // NBLK
    blk_s = gid_s % NBLK
    slot = base[blk_s] * BLK + pos            # slot within the core's stream

    # AG-table row (rank-ordered layout, layers 2/3)
    row23 = (s_own * PADS + s_loc)[order]
    dstl_s = dstl[order]

    g23 = np.zeros((P, T * BLK), np.int32)
    dv = np.full((P, T * BLK), -1.0, np.float32)
    flat = own_s * (T * BLK) + slot
    g23.reshape(-1)[flat] = row23.astype(np.int32)
    dv.reshape(-1)[flat] = dstl_s

    # layer-1 table row: per-core permuted layout, own shard first
    g1 = np.zeros((P, T * BLK), np.int32)
    s_own_s = s_own[order]
    s_loc_s = s_loc[order]
    for k in range(P):
        sel = own_s == k
        so = s_own_s[sel]
        # position of shard `so` in core k's permuted order [k, 0,1,..(!k)..,7]
        rank = np.where(so == k, 0, 1 + so - (so > k))
        g1.reshape(-1)[flat[sel]] = (rank * PADS + s_loc_s[sel]).astype(np.int32)

    x_pad = np.zeros((P, PADS, IN_DIM), np.float32)
    deg_pad = np.ones((P, PADS), np.float32)
    x_pad[newid // PADS, newid % PADS] = x
    deg_pad[newid // PADS, newid % PADS] = deg

    per_core = []
    for k in range(P):
        perm = [k] + [c for c in range(P) if c != k]
        xp = x_pad[perm].reshape(TBL, IN_DIM)
        degp = deg_pad[perm].reshape(P * NBLK, BLK).T   # [128, 392]
        per_core.append(dict(
            xpt=np.ascontiguousarray(xp.T),
            degp=np.ascontiguousarray(degp),
            g1=np.ascontiguousarray(g1[k].reshape(T, BLK).T.astype(np.int32)),
            g23=np.ascontiguousarray(g23[k].reshape(T, BLK).T.astype(np.int32)),
            dstl=np.ascontiguousarray(dv[k].reshape(T, BLK).T),
        ))
    return per_core, tuple(int(c) for c in C_arr), newid


# ------------------------------------------------------------- device build
def _build(C_arr):
    T = int(sum(C_arr))
    c_base = [0]
    for c in C_arr[:-1]:
        c_base.append(c_base[-1] + c)
    NFULL = P * NBLK          # 392 blocks in the full node space

    nc = bacc.Bacc("TRN2", target_bir_lowering=False, debug=False,
                   enable_asserts=False, num_devices=P,
                   dynamic_dma_scratch_size=65536)

    xpt_d = nc.dram_tensor("xpt", [IN_DIM, TBL], F32, kind="ExternalInput").ap()
    degp_d = nc.dram_tensor("degp", [BLK, NFULL], F32, kind="ExternalInput").ap()
    g1_d = nc.dram_tensor("g1", [BLK, T], I32, kind="ExternalInput").ap()
    g23_d = nc.dram_tensor("g23", [BLK, T], I32, kind="ExternalInput").ap()
    dstl_d = nc.dram_tensor("dstl", [BLK, T], F32, kind="ExternalInput").ap()
    w_d = [nc.dram_tensor(f"w{i}", [d, HID if i < 3 else OUT_DIM], F32,
                          kind="ExternalInput").ap()
           for i, d in enumerate([IN_DIM, HID, HID, HID])]
    bt_d = [nc.dram_tensor(f"bt{i}", [BLK, HID if i < 3 else OUT_DIM], F32,
                           kind="ExternalInput").ap()
            for i in range(4)]
    iota_d = nc.dram_tensor("iota", [BLK, BLK], F32, kind="ExternalInput").ap()
    iden_d = nc.dram_tensor("iden", [BLK, BLK], F32, kind="ExternalInput").ap()
    out_d = nc.dram_tensor("probs", [PADS, OUT_DIM], F32, kind="ExternalOutput").ap()

    rg = [list(range(P))]

    with tile.TileContext(nc) as tc:
        with (
            tc.tile_pool(name="const", bufs=1) as cp,
            tc.tile_pool(name="xin", bufs=3) as xp_pool,
            tc.tile_pool(name="ht", bufs=3) as hp,
            tc.tile_pool(name="zt", bufs=3) as zp,
            tc.tile_pool(name="oh", bufs=12) as ohp,
            tc.tile_pool(name="msg", bufs=32) as mp,
            tc.tile_pool(name="fin", bufs=2) as fp,
            tc.tile_pool(name="pstp", bufs=2, space="PSUM") as pstp,
            tc.tile_pool(name="psacc", bufs=4, space="PSUM") as psacc,
            tc.tile_pool(name="dram", bufs=1, space="DRAM") as dp,
        ):
            # ---- constants into SBUF
            w_sb, bt_sb = [], []
            for i in range(4):
                wt = cp.tile(list(w_d[i].shape), F32, tag=f"w{i}", name=f"w{i}")
                nc.sync.dma_start(wt[:], w_d[i])
                w_sb.append(wt)
                bt = cp.tile(list(bt_d[i].shape), F32, tag=f"bt{i}", name=f"bt{i}")
                nc.sync.dma_start(bt[:], bt_d[i])
                bt_sb.append(bt)
            iota_sb = cp.tile([BLK, BLK], F32, tag="iota")
            nc.sync.dma_start(iota_sb[:], iota_d)
            iden_sb = cp.tile([BLK, BLK], F32, tag="iden")
            nc.sync.dma_start(iden_sb[:], iden_d)
            g1_sb = cp.tile([BLK, T], I32, tag="g1")
            nc.sync.dma_start(g1_sb[:], g1_d)
            g23_sb = cp.tile([BLK, T], I32, tag="g23")
            nc.sync.dma_start(g23_sb[:], g23_d)
            dstl_sb = cp.tile([BLK, T], F32, tag="dstl")
            nc.sync.dma_start(dstl_sb[:], dstl_d)

            deg_sb = cp.tile([BLK, NFULL], F32, tag="deg")
            nc.sync.dma_start(deg_sb[:], degp_d)
            dinv_sb = cp.tile([BLK, NFULL], F32, tag="dinv")
            nc.vector.reciprocal(dinv_sb[:], deg_sb[:])
            nc.scalar.activation(dinv_sb[:], dinv_sb[:],
                                 mybir.ActivationFunctionType.Sqrt)

            h_sb = [cp.tile([BLK, NBLK * HID], F32, tag=f"h{i}", name=f"h{i}")
                    for i in range(2)]
            zt_own = cp.tile([BLK, NBLK * HID], F32, tag="zt_own")

            def transform_block(src_ap, d_in, w_t, b, zdst):
                """z~_block = dinv[:,b] * (src_block @ W) -> zdst [128, HID]"""
                tp_ps = pstp.tile([d_in, BLK], F32, tag="tp", name="tp")
                nc.tensor.transpose(tp_ps[:], src_ap, iden_sb[:])
                hT = hp.tile([d_in, BLK], F32, tag="hT", name="hT")
                nc.vector.tensor_copy(hT[:], tp_ps[:])
                z_ps = psacc.tile([BLK, HID], F32, tag="acc", name="z_ps")
                nc.tensor.matmul(z_ps[:], hT[:], w_t[:], start=True, stop=True)
                nc.vector.tensor_scalar(zdst, z_ps[:], dinv_sb[:, b:b + 1],
                                        None, mybir.AluOpType.mult)

            def readout_block(h_ap, b):
                tp_ps = pstp.tile([HID, BLK], F32, tag="tp", name="tp")
                nc.tensor.transpose(tp_ps[:], h_ap, iden_sb[:])
                hT = hp.tile([HID, BLK], F32, tag="hT", name="hT")
                nc.vector.tensor_copy(hT[:], tp_ps[:])
                o_ps = psacc.tile([BLK, OUT_DIM], F32, tag="acc", name="o_ps")
                nc.tensor.matmul(o_ps[:], hT[:], w_sb[3][:],
                                 start=True, stop=True)
                logit = fp.tile([BLK, OUT_DIM], F32, tag="logit", name="logit")
                nc.vector.tensor_tensor(logit[:], o_ps[:], bt_sb[3][:],
                                        mybir.AluOpType.add)
                nmx = fp.tile([BLK, 1], F32, tag="nmx", name="nmx")
                nc.vector.reduce_max(nmx[:], logit[:],
                                     axis=mybir.AxisListType.X, negate=True)
                ex = fp.tile([BLK, OUT_DIM], F32, tag="ex", name="ex")
                ssum = fp.tile([BLK, 1], F32, tag="ssum", name="ssum")
                nc.scalar.activation(ex[:], logit[:],
                                     mybir.ActivationFunctionType.Exp,
                                     bias=nmx[:], accum_out=ssum[:])
                rs = fp.tile([BLK, 1], F32, tag="rs", name="rs")
                nc.vector.reciprocal(rs[:], ssum[:])
                prob = fp.tile([BLK, OUT_DIM], F32, tag="prob", name="prob")
                nc.vector.tensor_scalar(prob[:], ex[:], rs[:], None,
                                        mybir.AluOpType.mult)
                nc.sync.dma_start(out_d[b * BLK:(b + 1) * BLK, :], prob[:])

            def propagate(gidx_sb, table, h_nxt, b_t, readout=False):
                for b in range(NBLK):
                    C_b = C_arr[b]
                    agg_ps = psacc.tile([BLK, HID], F32, tag="acc", name="agg_ps")
                    for c in range(C_b):
                        t = c_base[b] + c
                        msg = mp.tile([BLK, HID], F32, tag="msg", name="msg")
                        nc.gpsimd.indirect_dma_start(
                            out=msg[:], out_offset=None, in_=table[:],
                            in_offset=bass.IndirectOffsetOnAxis(
                                ap=gidx_sb[:, t:t + 1], axis=0))
                        oh = ohp.tile([BLK, BLK], F32, tag="oh", name="oh")
                        nc.vector.tensor_scalar(
                            oh[:], iota_sb[:], dstl_sb[:, t:t + 1], None,
                            mybir.AluOpType.is_equal)
                        nc.tensor.matmul(agg_ps[:], oh[:], msg[:],
                                         start=(c == 0), stop=(c == C_b - 1))
                    sl = slice(b * HID, (b + 1) * HID)
                    tot = zp.tile([BLK, HID], F32, tag="tot", name="tot")
                    nc.vector.tensor_tensor(tot[:], agg_ps[:], zt_own[:, sl],
                                            mybir.AluOpType.add)
                    nc.vector.scalar_tensor_tensor(
                        h_nxt[:, sl], tot[:], dinv_sb[:, b:b + 1], b_t[:],
                        mybir.AluOpType.mult, mybir.AluOpType.add)
                    nc.scalar.activation(h_nxt[:, sl], h_nxt[:, sl],
                                         mybir.ActivationFunctionType.Relu)
                    if readout:
                        readout_block(h_nxt[:, sl], b)

            # ---------------- layer 1: full local table (x replicated)
            # batched 8-block staging keeps the sync engine off the critical
            # path (one 512KB load + one 256KB store per 8 blocks)
            table1 = dp.tile([TBL, HID], F32, tag="tbl0")
            GB = 8
            for g in range(NFULL // GB):
                # x arrives pre-transposed: columns are nodes, so each block
                # slice is directly the matmul's stationary operand
                xg = xp_pool.tile([IN_DIM, GB * BLK], F32, tag="xb", name="xb")
                nc.sync.dma_start(xg[:], xpt_d[:, g * GB * BLK:(g + 1) * GB * BLK])
                zg = zp.tile([BLK, GB * HID], F32, tag="zd", name="zd")
                for j in range(GB):
                    b = g * GB + j
                    z_ps = psacc.tile([BLK, HID], F32, tag="acc", name="z_ps")
                    nc.tensor.matmul(z_ps[:], xg[:, j * BLK:(j + 1) * BLK],
                                     w_sb[0][:], start=True, stop=True)
                    nc.vector.tensor_scalar(zg[:, j * HID:(j + 1) * HID],
                                            z_ps[:], dinv_sb[:, b:b + 1],
                                            None, mybir.AluOpType.mult)
                    if b < NBLK:
                        nc.vector.tensor_copy(
                            zt_own[:, b * HID:(b + 1) * HID],
                            zg[:, j * HID:(j + 1) * HID])
                nc.sync.dma_start(
                    table1[g * GB * BLK:(g + 1) * GB * BLK, :].rearrange(
                        "(j p) f -> p j f", p=BLK),
                    zg[:].rearrange("p (j f) -> p j f", f=HID))
            propagate(g1_sb, table1, h_sb[0], bt_sb[0])

            # ---------------- layers 2, 3: shard transform + AllGather
            for li in (1, 2):
                h_cur = h_sb[(li + 1) % 2]
                h_nxt = h_sb[li % 2]
                for b in range(NBLK):
                    transform_block(h_cur[:, b * HID:(b + 1) * HID], HID,
                                    w_sb[li], b,
                                    zt_own[:, b * HID:(b + 1) * HID])
                ag_in = dp.tile([PADS, HID], F32, tag=f"agin{li}",
                                name=f"agin{li}")
                nc.sync.dma_start(
                    ag_in[:].rearrange("(b p) f -> p b f", p=BLK),
                    zt_own[:].rearrange("p (b f) -> p b f", f=HID))
                table = dp.tile([TBL, HID], F32, tag=f"tbl{li}",
                                name=f"table{li}", addr_space="Shared")
                nc.gpsimd.collective_compute(
                    "AllGather", mybir.AluOpType.bypass, replica_groups=rg,
                    ins=[ag_in.opt()], outs=[table.opt()])
                propagate(g23_sb, table, h_nxt, bt_sb[li], readout=(li == 2))

    nc.compile()
    return nc


# ------------------------------------------------------------- entry point
_CACHE = {}


def _get_program(C_arr):
    if C_arr not in _CACHE:
        _CACHE[C_arr] = _build(C_arr)
    return _CACHE[C_arr]


def kernel(x, edge_index, W1, b1, W2, b2, W3, b3, Wr, br, trace=False):
    per_core, C_arr, newid = _preprocess(x, edge_index)
    nc = _get_program(C_arr)

    ws = [np.asarray(w, np.float32) for w in (W1, W2, W3, Wr)]
    bts = [np.tile(np.asarray(b, np.float32).reshape(1, -1), (BLK, 1))
           for b in (b1, b2, b3, br)]
    iota = np.tile(np.arange(BLK, dtype=np.float32), (BLK, 1))
    iden = np.eye(BLK, dtype=np.float32)

    in_maps = []
    for k in range(P):
        m = dict(per_core[k])
        for i in range(4):
            m[f"w{i}"] = ws[i]
            m[f"bt{i}"] = bts[i]
        m["iota"] = iota
        m["iden"] = iden
        in_maps.append(m)

    res = run_bass_kernel_spmd(nc, in_maps, core_ids=list(range(P)),
                               trace=trace)
    allp = np.concatenate([res.results[k]["probs"] for k in range(P)], axis=0)
    out = allp[newid]
    kernel.last_results = res
    return out



# revision 6
# speedup vs baseline: 1.2689x; 1.2689x over previous
"""GCN (3-layer + readout) on 8 Trainium2 NeuronCores.

Strategy (dst-node sharding, 1D graph parallel):
  - Nodes are LPT-packed by in-degree into 8*49 = 392 blocks of 128 (8 cores
    x 49 blocks, 6272 padded rows/core).  Each core aggregates messages for
    the edges whose dst lands in its shard.
  - Per layer: each core transforms its own shard z = h @ W (PE, bf16),
    scales rows by dinv = deg^-1/2, writes duplicated bf16 rows
    [zt | zt] (256B) into a shard table, AllGathers the full
    [50176, 128] bf16 table (all three layers, incl. layer 1 from x).
  - Edge gathers: batched InstDMAGatherAnt (SWDGE) — ONE instruction gathers
    thousands of 256B rows (vs one 128-row indirect DMA per chunk), keyed by
    int16 row indices.  Rows >= 32768 exceed int16, so every (core, block)
    edge list is split into a lo stream (row < 32768, gathered from
    table[0:]) and a hi stream (gathered from table[32768:] with idx-32768).
  - Scatter-add on the TensorEngine: per 128-edge chunk a one-hot
    onehot[e, d] = (dst_slot[e] == d) is built in bf16 with one DVE
    tensor_tensor(is_equal) per (block, stream) using broadcast APs;
    psum[128, 64] += onehot^T @ msg accumulates the dst-block aggregate.
    bf16 operands make every scatter matmul a single PE pass.
  - Self loops: zt_own (fp32) kept in SBUF, added before the dst-side dinv.
  - Host-side preprocessing is strictly index/metadata work (edge bucketing,
    padding, degree counting); all float math runs on device.
"""

import numpy as np
import ml_dtypes

from concourse import bacc, bass, mybir, tile
from concourse.bass_utils import run_bass_kernel_spmd

# ---------------------------------------------------------------- constants
P = 8                      # cores
N = 50000                  # nodes
IN_DIM = 128
HID = 64
OUT_DIM = 10
BLK = 128
HALF = 32768               # int16 index limit for dma_gather

SHARD = N // P             # 6250
NBLK = (SHARD + BLK - 1) // BLK   # 49
PADS = NBLK * BLK          # 6272
TBL = P * PADS             # 50176

G = 7                      # dst blocks per gather group (49 = 7*7)
MAXCH = 8                  # max chunks (128 idxs each) per gather instruction
                           # (SWDGE desc ring: >=1536-idx gathers crash, 1024 ok)

F32 = mybir.dt.float32
BF16 = mybir.dt.bfloat16
I16 = mybir.dt.int16

BFNP = ml_dtypes.bfloat16


# ------------------------------------------------------------- layout
def _layout(C_lo, C_hi):
    """Global chunk-column layout: per group g, [lo chunks of blocks
    g*G..][hi chunks of same blocks].  Returns per-(block,stream) chunk
    bases and per-group info."""
    lo_base = {}
    hi_base = {}
    groups = []
    col = 0
    for g in range((NBLK + G - 1) // G):
        bs = list(range(g * G, min(NBLK, (g + 1) * G)))
        g_start = col
        for b in bs:
            lo_base[b] = col
            col += C_lo[b]
        nlo = col - g_start
        for b in bs:
            hi_base[b] = col
            col += C_hi[b]
        nhi = col - g_start - nlo
        groups.append(dict(bs=bs, start=g_start, nlo=nlo, nhi=nhi))
    return col, lo_base, hi_base, groups


# ------------------------------------------------------------- host prep
def _preprocess(x, edge_index):
    """Bucket edges into per-(core, dst-block) chunk streams, split lo/hi
    by src table row (int16 gather index range)."""
    import heapq

    x = np.asarray(x, np.float32)
    ei = np.asarray(edge_index, np.int64)
    src, dst = ei[0], ei[1]

    degE = np.bincount(dst, minlength=N).astype(np.int64)
    deg = (degE + 1).astype(np.float32)

    # LPT pack nodes into the P*NBLK bins by in-degree (capacity 128/bin)
    NBINS = P * NBLK
    order_n = np.argsort(-degE, kind="stable")
    heap = [(0, b) for b in range(NBINS)]
    heapq.heapify(heap)
    fill = np.zeros(NBINS, np.int64)
    node_bin = np.empty(N, np.int64)
    node_slot = np.empty(N, np.int64)
    for n in order_n:
        while True:
            s, b = heapq.heappop(heap)
            if fill[b] < BLK:
                break
        node_bin[n] = b
        node_slot[n] = fill[b]
        fill[b] += 1
        heapq.heappush(heap, (s + int(degE[n]), b))

    newid = node_bin * BLK + node_slot          # padded global table row

    e_core = node_bin[dst] // NBLK
    e_blk = node_bin[dst] % NBLK
    e_dslot = node_slot[dst]
    e_srow = newid[src]
    e_hi = (e_srow >= HALF).astype(np.int64)

    # per (core, blk, stream) counts -> uniform chunk counts (max over cores)
    key = ((e_core * NBLK + e_blk) * 2 + e_hi)
    cnt = np.bincount(key, minlength=P * NBLK * 2).reshape(P, NBLK, 2)
    cmax = cnt.max(axis=0)                      # [NBLK, 2]
    C_lo = tuple(int(c) for c in np.ceil(cmax[:, 0] / BLK).astype(np.int64))
    C_hi = tuple(int(c) for c in np.ceil(cmax[:, 1] / BLK).astype(np.int64))

    T, lo_base, hi_base, groups = _layout(C_lo, C_hi)

    # sort edges by (core, blk, stream); fill per-core index/dstl arrays
    order = np.lexsort((e_hi, e_blk, e_core))
    s_core = e_core[order]
    s_blk = e_blk[order]
    s_hi = e_hi[order]
    s_srow = e_srow[order]
    s_dslot = e_dslot[order]

    skey = (s_core * NBLK + s_blk) * 2 + s_hi
    scounts = np.bincount(skey, minlength=P * NBLK * 2)
    sstarts = np.concatenate([[0], np.cumsum(scounts)[:-1]])
    pos = np.arange(skey.size) - sstarts[skey]

    base_arr = np.empty((NBLK, 2), np.int64)
    for b in range(NBLK):
        base_arr[b, 0] = lo_base[b]
        base_arr[b, 1] = hi_base[b]
    # stream slot of each edge within its core's [T*128] chunk stream
    slot = base_arr[s_blk, s_hi] * BLK + pos
    idxval = np.where(s_hi == 1, s_srow - HALF, s_srow)

    gidx_all = np.zeros((P, T * BLK), np.int16)
    dstl_all = np.full((P, T * BLK), -1.0, np.float32)
    flat = s_core * (T * BLK) + slot
    gidx_all.reshape(-1)[flat] = idxval.astype(np.int16)
    dstl_all.reshape(-1)[flat] = s_dslot.astype(np.float32)

    # wrap indices: chunk c, pos p -> partition p%16, col 8c + p//16
    gidx_w = (gidx_all.reshape(P, T, 8, 16).transpose(0, 3, 1, 2)
              .reshape(P, 16, T * 8))
    gidx_w = np.tile(gidx_w, (1, 8, 1))          # replicate to 128 partitions
    dstl_w = dstl_all.reshape(P, T, BLK).transpose(0, 2, 1)  # [P, 128, T]

    x_pad = np.zeros((P, PADS, IN_DIM), np.float32)
    deg_pad = np.ones((P, PADS), np.float32)
    x_pad[newid // PADS, newid % PADS] = x
    deg_pad[newid // PADS, newid % PADS] = deg

    per_core = []
    for k in range(P):
        per_core.append(dict(
            xpt=np.ascontiguousarray(x_pad[k].T),                 # [128, PADS]
            degp=np.ascontiguousarray(
                deg_pad[k].reshape(NBLK, BLK).T),                 # [128, NBLK]
            gidx=np.ascontiguousarray(gidx_w[k]),                 # [128, 8T] i16
            dstl=np.ascontiguousarray(dstl_w[k].astype(BFNP)),    # [128, T] bf16
        ))
    return per_core, C_lo, C_hi, newid


# ------------------------------------------------------------- device build
def _build(C_lo, C_hi):
    T, lo_base, hi_base, groups = _layout(C_lo, C_hi)
    CHMAX = max(g["nlo"] + g["nhi"] for g in groups)
    CLO_MAX = max(max(C_lo), 1)
    CHI_MAX = max(max(C_hi), 1)

    nc = bacc.Bacc("TRN2", target_bir_lowering=False, debug=False,
                   enable_asserts=False, num_devices=P,
                   dynamic_dma_scratch_size=65536)

    xpt_d = nc.dram_tensor("xpt", [IN_DIM, PADS], F32, kind="ExternalInput").ap()
    degp_d = nc.dram_tensor("degp", [BLK, NBLK], F32, kind="ExternalInput").ap()
    gidx_d = nc.dram_tensor("gidx", [BLK, 8 * T], I16, kind="ExternalInput").ap()
    dstl_d = nc.dram_tensor("dstl", [BLK, T], BF16, kind="ExternalInput").ap()
    w_d = [nc.dram_tensor(f"w{i}", [d, HID if i < 3 else OUT_DIM], F32,
                          kind="ExternalInput").ap()
           for i, d in enumerate([IN_DIM, HID, HID, HID])]
    bt_d = [nc.dram_tensor(f"bt{i}", [BLK, HID if i < 3 else OUT_DIM], F32,
                           kind="ExternalInput").ap()
            for i in range(4)]
    iota_d = nc.dram_tensor("iota", [BLK, BLK], BF16, kind="ExternalInput").ap()
    iden_d = nc.dram_tensor("iden", [BLK, BLK], BF16, kind="ExternalInput").ap()
    out_d = nc.dram_tensor("probs", [PADS, OUT_DIM], F32,
                           kind="ExternalOutput").ap()

    rg = [list(range(P))]

    with tile.TileContext(nc) as tc:
        with (
            tc.tile_pool(name="const", bufs=1) as cp,
            tc.tile_pool(name="xin", bufs=2) as xp_pool,
            tc.tile_pool(name="ht", bufs=3) as hp,
            tc.tile_pool(name="ztb", bufs=3) as zp,
            tc.tile_pool(name="oh", bufs=4) as ohp,
            tc.tile_pool(name="msg", bufs=2) as mp,
            tc.tile_pool(name="fin", bufs=4) as fp,
            tc.tile_pool(name="pstp", bufs=2, space="PSUM") as pstp,
            tc.tile_pool(name="psz", bufs=2, space="PSUM") as psz,
            tc.tile_pool(name="psacc", bufs=4, space="PSUM") as psacc,
            tc.tile_pool(name="dram", bufs=1, space="DRAM") as dp,
        ):
            # ---- constants into SBUF
            w_sb, bt_sb = [], []
            for i in range(4):
                wt = cp.tile(list(w_d[i].shape), F32, tag=f"w{i}", name=f"w{i}")
                nc.sync.dma_start(wt[:], w_d[i])
                w_sb.append(wt)
                bt = cp.tile(list(bt_d[i].shape), F32, tag=f"bt{i}", name=f"bt{i}")
                nc.sync.dma_start(bt[:], bt_d[i])
                bt_sb.append(bt)
            # bf16 weight copies for layers 2/3 + readout
            wb_sb = []
            for i in (1, 2, 3):
                wb = cp.tile(list(w_d[i].shape), BF16, tag=f"wb{i}", name=f"wb{i}")
                nc.vector.tensor_copy(wb[:], w_sb[i][:])
                wb_sb.append(wb)
            wb_sb = {1: wb_sb[0], 2: wb_sb[1], 3: wb_sb[2]}

            iota_sb = cp.tile([BLK, BLK], BF16, tag="iota")
            nc.sync.dma_start(iota_sb[:], iota_d)
            iden_sb = cp.tile([BLK, BLK], BF16, tag="iden")
            nc.sync.dma_start(iden_sb[:], iden_d)
            gidx_sb = cp.tile([BLK, 8 * T], I16, tag="gidx")
            nc.sync.dma_start(gidx_sb[:], gidx_d)
            dstl_sb = cp.tile([BLK, T], BF16, tag="dstl")
            nc.sync.dma_start(dstl_sb[:], dstl_d)

            deg_sb = cp.tile([BLK, NBLK], F32, tag="deg")
            nc.sync.dma_start(deg_sb[:], degp_d)
            dinv_sb = cp.tile([BLK, NBLK], F32, tag="dinv")
            nc.vector.reciprocal(dinv_sb[:], deg_sb[:])
            nc.scalar.activation(dinv_sb[:], dinv_sb[:],
                                 mybir.ActivationFunctionType.Sqrt)

            zt_own = cp.tile([BLK, NBLK * HID], F32, tag="zt_own")
            h_sb = [cp.tile([BLK, NBLK * HID], BF16, tag=f"h{i}", name=f"h{i}")
                    for i in range(2)]

            ag = [dp.tile([PADS, 2 * HID], BF16, tag=f"ag{l}", name=f"ag{l}")
                  for l in range(3)]
            tables = [dp.tile([TBL, 2 * HID], BF16, tag=f"tbl{l}",
                              name=f"table{l}", addr_space="Shared")
                      for l in range(3)]

            def stage_table_row(b, z_ps, l_next):
                """zt_own[:,b] = dinv*z;  ag[l_next] rows b*128.. = [zt|zt] bf16"""
                sl = slice(b * HID, (b + 1) * HID)
                nc.vector.tensor_scalar(zt_own[:, sl], z_ps[:],
                                        dinv_sb[:, b:b + 1], None,
                                        mybir.AluOpType.mult)
                ztb = zp.tile([BLK, 2 * HID], BF16, tag="ztb", name="ztb")
                nc.vector.tensor_copy(
                    ztb[:].rearrange("p (d e) -> p d e", e=HID),
                    zt_own[:, sl].unsqueeze(1).broadcast_to([BLK, 2, HID]))
                nc.sync.dma_start(ag[l_next][b * BLK:(b + 1) * BLK, :], ztb[:])

            def transform_block(b, h_cur, l_next):
                """layer l_next table shard from h_cur block b (bf16)"""
                tp_ps = pstp.tile([HID, BLK], BF16, tag="tp", name="tp")
                nc.tensor.transpose(tp_ps[:], h_cur[:, b * HID:(b + 1) * HID],
                                    iden_sb[:])
                hT = hp.tile([HID, BLK], BF16, tag="hT", name="hT")
                nc.vector.tensor_copy(hT[:], tp_ps[:])
                z_ps = psz.tile([BLK, HID], F32, tag="z", name="z_ps")
                nc.tensor.matmul(z_ps[:], hT[:], wb_sb[l_next][:],
                                 start=True, stop=True)
                stage_table_row(b, z_ps, l_next)

            def readout_block(h_cur, b):
                tp_ps = pstp.tile([HID, BLK], BF16, tag="tp", name="tp")
                nc.tensor.transpose(tp_ps[:], h_cur[:, b * HID:(b + 1) * HID],
                                    iden_sb[:])
                hT = hp.tile([HID, BLK], BF16, tag="hT", name="hT")
                nc.vector.tensor_copy(hT[:], tp_ps[:])
                o_ps = psz.tile([BLK, HID], F32, tag="z", name="o_ps")
                nc.tensor.matmul(o_ps[:, :OUT_DIM], hT[:], wb_sb[3][:],
                                 start=True, stop=True)
                logit = fp.tile([BLK, OUT_DIM], F32, tag="logit", name="logit")
                nc.vector.tensor_tensor(logit[:], o_ps[:, :OUT_DIM], bt_sb[3][:],
                                        mybir.AluOpType.add)
                nmx = fp.tile([BLK, 1], F32, tag="nmx", name="nmx")
                nc.vector.reduce_max(nmx[:], logit[:],
                                     axis=mybir.AxisListType.X, negate=True)
                ex = fp.tile([BLK, OUT_DIM], F32, tag="ex", name="ex")
                ssum = fp.tile([BLK, 1], F32, tag="ssum", name="ssum")
                nc.scalar.activation(ex[:], logit[:],
                                     mybir.ActivationFunctionType.Exp,
                                     bias=nmx[:], accum_out=ssum[:])
                rs = fp.tile([BLK, 1], F32, tag="rs", name="rs")
                nc.vector.reciprocal(rs[:], ssum[:])
                prob = fp.tile([BLK, OUT_DIM], F32, tag="prob", name="prob")
                nc.vector.tensor_scalar(prob[:], ex[:], rs[:], None,
                                        mybir.AluOpType.mult)
                nc.sync.dma_start(out_d[b * BLK:(b + 1) * BLK, :], prob[:])

            def emit_onehot(oh, cmax_ch, base, C):
                nc.vector.tensor_tensor(
                    oh[:, :C * BLK].rearrange("p (c e) -> p c e", e=BLK),
                    iota_sb[:].unsqueeze(1).broadcast_to([BLK, C, BLK]),
                    dstl_sb[:, base:base + C].unsqueeze(2)
                        .broadcast_to([BLK, C, BLK]),
                    mybir.AluOpType.is_equal)

            # ---------------- layer-0 transform (x shard, streamed)
            for g in groups:
                bs = g["bs"]
                xg = xp_pool.tile([IN_DIM, G * BLK], F32, tag="xb", name="xb")
                nc.sync.dma_start(xg[:, :len(bs) * BLK],
                                  xpt_d[:, bs[0] * BLK:(bs[-1] + 1) * BLK])
                for j, b in enumerate(bs):
                    z_ps = psz.tile([BLK, HID], F32, tag="z", name="z_ps")
                    nc.tensor.matmul(z_ps[:], xg[:, j * BLK:(j + 1) * BLK],
                                     w_sb[0][:], start=True, stop=True)
                    stage_table_row(b, z_ps, 0)
            nc.gpsimd.collective_compute(
                "AllGather", mybir.AluOpType.bypass, replica_groups=rg,
                ins=[ag[0].opt()], outs=[tables[0].opt()])

            # ---------------- layers 0..2: gather + scatter (+ next transform)
            for l in range(3):
                h_nxt = h_sb[l % 2]
                table_t = tables[l]
                for g in groups:
                    bs, g_start = g["bs"], g["start"]
                    nlo, nhi = g["nlo"], g["nhi"]
                    msg = mp.tile([BLK, CHMAX * 2 * HID], BF16,
                                  tag="msg", name="msg")
                    # batched gathers: lo from table[0:], hi from table[HALF:]
                    col = 0
                    for (n_ch, src_off) in ((nlo, 0), (nhi, HALF)):
                        done = 0
                        while done < n_ch:
                            nch = min(MAXCH, n_ch - done)
                            c0 = g_start + col
                            nc.gpsimd.dma_gather(
                                msg[:, col * 2 * HID:(col + nch) * 2 * HID]
                                    .rearrange("p (c e) -> p c e", e=2 * HID),
                                table_t[src_off:, :],
                                gidx_sb[:, 8 * c0:8 * (c0 + nch)],
                                num_idxs=nch * BLK,
                                num_idxs_reg=nch * BLK,
                                elem_size=2 * HID)
                            done += nch
                            col += nch
                    for b in bs:
                        n_tot = C_lo[b] + C_hi[b]
                        agg_ps = psacc.tile([BLK, HID], F32, tag="acc",
                                            name="agg_ps")
                        k = 0
                        for (C, cmax, base) in ((C_lo[b], CLO_MAX, lo_base[b]),
                                                (C_hi[b], CHI_MAX, hi_base[b])):
                            if C == 0:
                                continue
                            oh = ohp.tile([BLK, cmax * BLK], BF16,
                                          tag=f"oh{cmax}", name="oh")
                            emit_onehot(oh, cmax, base, C)
                            for c in range(C):
                                mc = base - g_start + c
                                nc.tensor.matmul(
                                    agg_ps[:],
                                    oh[:, c * BLK:(c + 1) * BLK],
                                    msg[:, mc * 2 * HID:mc * 2 * HID + HID],
                                    start=(k == 0), stop=(k == n_tot - 1))
                                k += 1
                        sl = slice(b * HID, (b + 1) * HID)
                        tot = fp.tile([BLK, HID], F32, tag="tot", name="tot")
                        nc.vector.tensor_tensor(tot[:], agg_ps[:], zt_own[:, sl],
                                                mybir.AluOpType.add)
                        pre = fp.tile([BLK, HID], F32, tag="pre", name="pre")
                        nc.vector.scalar_tensor_tensor(
                            pre[:], tot[:], dinv_sb[:, b:b + 1], bt_sb[l][:],
                            mybir.AluOpType.mult, mybir.AluOpType.add)
                        nc.scalar.activation(h_nxt[:, sl], pre[:],
                                             mybir.ActivationFunctionType.Relu)
                        if l < 2:
                            transform_block(b, h_nxt, l + 1)
                        else:
                            readout_block(h_nxt, b)
                if l < 2:
                    nc.gpsimd.collective_compute(
                        "AllGather", mybir.AluOpType.bypass, replica_groups=rg,
                        ins=[ag[l + 1].opt()], outs=[tables[l + 1].opt()])

    nc.compile()
    return nc


# ------------------------------------------------------------- entry point
_CACHE = {}


def _get_program(key):
    if key not in _CACHE:
        _CACHE[key] = _build(*key)
    return _CACHE[key]


def _in_maps(per_core, W1, b1, W2, b2, W3, b3, Wr, br):
    ws = [np.asarray(w, np.float32) for w in (W1, W2, W3, Wr)]
    bts = [np.tile(np.asarray(b, np.float32).reshape(1, -1), (BLK, 1))
           for b in (b1, b2, b3, br)]
    iota = np.tile(np.arange(BLK, dtype=BFNP), (BLK, 1))
    iden = np.eye(BLK, dtype=BFNP)
    in_maps = []
    for k in range(P):
        m = dict(per_core[k])
        for i in range(4):
            m[f"w{i}"] = ws[i]
            m[f"bt{i}"] = bts[i]
        m["iota"] = iota
        m["iden"] = iden
        in_maps.append(m)
    return in_maps


def kernel(x, edge_index, W1, b1, W2, b2, W3, b3, Wr, br, trace=False):
    per_core, C_lo, C_hi, newid = _preprocess(x, edge_index)
    nc = _get_program((C_lo, C_hi))
    in_maps = _in_maps(per_core, W1, b1, W2, b2, W3, b3, Wr, br)
    res = run_bass_kernel_spmd(nc, in_maps, core_ids=list(range(P)),
                               trace=trace)
    allp = np.concatenate([res.results[k]["probs"] for k in range(P)], axis=0)
    out = allp[newid]
    kernel.last_results = res
    return out


# revision 7
# speedup vs baseline: 1.3010x; 1.0253x over previous
"""GCN (3-layer + readout) on 8 Trainium2 NeuronCores — v3.

Architecture (dst-node sharding, 1D graph parallel):
  - Nodes LPT-packed by in-degree into 8 cores x 49 blocks of 128.
  - Table rows renumbered into two halves aligned to the int16 gather-index
    limit: half A = blocks 0..31 of every core (rows 0..32767 exactly),
    half B = blocks 32..48 (rows 32768..50175).  Each half is a separate
    DRAM tensor filled by its own AllGather, so a layer's A-half gathers
    can start while the B-half collective still runs (pipelined AGs).
  - Per layer: shard transform z = h @ W (PE, bf16), rows scaled by
    dinv = deg^-1/2, duplicated bf16 rows [zt|zt] (256B) staged into
    agA/agB, AllGather-A fired after block 31's transform, -B after 48.
  - Gathers: batched InstDMAGatherAnt, one instruction per (dst-block,
    half, sub-range), int16 indices; per-core TRUE edge counts are loaded
    into a Q7 register (reg_load) so padded tail indices (-1) generate no
    descriptors — Q7 descriptor generation (~8.5ns/row) is the kernel's
    critical resource.
  - Scatter on PE: bf16 one-hot (DVE is_equal, broadcast APs) x gathered
    messages accumulate per-block aggregates in PSUM (single-pass bf16
    matmuls).
  - Self loops: fp32 zt_own in SBUF added before the dst-side dinv scale.
  - Host preprocessing is index/metadata work only.
"""

import numpy as np
import ml_dtypes

from concourse import bacc, bass, mybir, tile
from concourse.bass_utils import run_bass_kernel_spmd

# ---------------------------------------------------------------- constants
P = 8
N = 50000
IN_DIM = 128
HID = 64
OUT_DIM = 10
BLK = 128

SHARD = N // P             # 6250
NBLK = (SHARD + BLK - 1) // BLK   # 49
PADS = NBLK * BLK          # 6272
ABLK = 32                  # blocks per core in half A
BBLK = NBLK - ABLK         # 17
AROWS = P * ABLK * BLK     # 32768 == int16 limit
BROWS = P * BBLK * BLK     # 17408

G = 5                      # dst blocks per msg-tile group
SUBMAX = 8                 # max chunks per gather instruction (desc ring)
REGCNT = False             # reg-count skipping measured as a net loss

F32 = mybir.dt.float32
BF16 = mybir.dt.bfloat16
I16 = mybir.dt.int16
I32 = mybir.dt.int32

BFNP = ml_dtypes.bfloat16


# ------------------------------------------------------------- layout
def _layout(C_lo, C_hi):
    """Chunk-column layout: per group g of G blocks, [lo chunks of the
    blocks][hi chunks].  Gather instructions are per (block, stream),
    sub-split at SUBMAX chunks."""
    lo_base = {}
    hi_base = {}
    groups = []
    col = 0
    for g in range((NBLK + G - 1) // G):
        bs = list(range(g * G, min(NBLK, (g + 1) * G)))
        g_start = col
        for b in bs:
            lo_base[b] = col
            col += C_lo[b]
        nlo = col - g_start
        for b in bs:
            hi_base[b] = col
            col += C_hi[b]
        nhi = col - g_start - nlo
        groups.append(dict(bs=bs, start=g_start, nlo=nlo, nhi=nhi))
    return col, lo_base, hi_base, groups


def _instr_list(C_lo, C_hi):
    """Gather instructions: (block, stream, chunk_base, nch, sub_off).
    One stream of instrs per (block, stream) split at SUBMAX chunks."""
    T, lo_base, hi_base, groups = _layout(C_lo, C_hi)
    instrs = []
    for b in range(NBLK):
        for s, (C, base) in enumerate(((C_lo[b], lo_base[b]),
                                       (C_hi[b], hi_base[b]))):
            done = 0
            while done < C:
                nch = min(SUBMAX, C - done)
                instrs.append((b, s, base + done, nch, done))
                done += nch
    return instrs


# ------------------------------------------------------------- host prep
def _preprocess(x, edge_index):
    import heapq

    x = np.asarray(x, np.float32)
    ei = np.asarray(edge_index, np.int64)
    src, dst = ei[0], ei[1]

    degE = np.bincount(dst, minlength=N).astype(np.int64)
    deg = (degE + 1).astype(np.float32)

    NBINS = P * NBLK
    order_n = np.argsort(-degE, kind="stable")
    heap = [(0, b) for b in range(NBINS)]
    heapq.heapify(heap)
    fill = np.zeros(NBINS, np.int64)
    node_bin = np.empty(N, np.int64)
    node_slot = np.empty(N, np.int64)
    for n in order_n:
        while True:
            s, b = heapq.heappop(heap)
            if fill[b] < BLK:
                break
        node_bin[n] = b
        node_slot[n] = fill[b]
        fill[b] += 1
        heapq.heappush(heap, (s + int(degE[n]), b))

    newid = node_bin * BLK + node_slot        # core-concat output row
    k_of = node_bin // NBLK
    b_of = node_bin % NBLK
    # table row: half A (blocks 0..31) rows 0..32767, half B above
    trow = np.where(
        b_of < ABLK,
        k_of * (ABLK * BLK) + b_of * BLK + node_slot,
        AROWS + k_of * (BBLK * BLK) + (b_of - ABLK) * BLK + node_slot)

    e_core = k_of[dst]
    e_blk = b_of[dst]
    e_dslot = node_slot[dst]
    e_srow = trow[src]
    e_hi = (e_srow >= AROWS).astype(np.int64)

    key = ((e_core * NBLK + e_blk) * 2 + e_hi)
    cnt = np.bincount(key, minlength=P * NBLK * 2).reshape(P, NBLK, 2)
    cmax = cnt.max(axis=0)
    C_lo = tuple(int(c) for c in np.maximum(
        np.ceil(cmax[:, 0] / BLK).astype(np.int64), 1))
    C_hi = tuple(int(c) for c in np.maximum(
        np.ceil(cmax[:, 1] / BLK).astype(np.int64), 1))

    T, lo_base, hi_base, groups = _layout(C_lo, C_hi)
    instrs = _instr_list(C_lo, C_hi)

    order = np.lexsort((e_hi, e_blk, e_core))
    s_core = e_core[order]
    s_blk = e_blk[order]
    s_hi = e_hi[order]
    s_srow = e_srow[order]
    s_dslot = e_dslot[order]

    skey = (s_core * NBLK + s_blk) * 2 + s_hi
    scounts = np.bincount(skey, minlength=P * NBLK * 2)
    sstarts = np.concatenate([[0], np.cumsum(scounts)[:-1]])
    pos = np.arange(skey.size) - sstarts[skey]

    base_arr = np.empty((NBLK, 2), np.int64)
    for b in range(NBLK):
        base_arr[b, 0] = lo_base[b]
        base_arr[b, 1] = hi_base[b]
    slot = base_arr[s_blk, s_hi] * BLK + pos
    idxval = np.where(s_hi == 1, s_srow - AROWS, s_srow)

    # pad indices are -1 (skipped by the gather when REGCNT) except the
    # first slot of each instruction, which must stay valid (>=0).
    gidx_all = np.full((P, T * BLK), -1, np.int16)
    dstl_all = np.full((P, T * BLK), -1.0, np.float32)
    flat = s_core * (T * BLK) + slot
    gidx_all.reshape(-1)[flat] = idxval.astype(np.int16)
    dstl_all.reshape(-1)[flat] = s_dslot.astype(np.float32)

    # per-core per-instruction valid counts (clamped to [1, nch*128])
    NI = len(instrs)
    cnts = np.zeros((P, NI), np.int32)
    for i, (b, s, cbase, nch, soff) in enumerate(instrs):
        c = cnt[:, b, s] - soff * BLK
        cnts[:, i] = np.clip(c, 1, nch * BLK)
        # ensure slot 0 of this instruction is valid for every core
        p0 = cbase * BLK
        colv = gidx_all[:, p0]
        gidx_all[:, p0] = np.where(colv < 0, 0, colv)
    if not REGCNT:
        gidx_all = np.maximum(gidx_all, 0)

    gidx_w = (gidx_all.reshape(P, T, 8, 16).transpose(0, 3, 1, 2)
              .reshape(P, 16, T * 8))
    gidx_w = np.tile(gidx_w, (1, 8, 1))
    dstl_w = dstl_all.reshape(P, T, BLK).transpose(0, 2, 1)

    x_pad = np.zeros((P, PADS, IN_DIM), np.float32)
    deg_pad = np.ones((P, PADS), np.float32)
    x_pad[newid // PADS, newid % PADS] = x
    deg_pad[newid // PADS, newid % PADS] = deg

    per_core = []
    for k in range(P):
        per_core.append(dict(
            xpt=np.ascontiguousarray(x_pad[k].T),
            degp=np.ascontiguousarray(deg_pad[k].reshape(NBLK, BLK).T),
            gidx=np.ascontiguousarray(gidx_w[k]),
            dstl=np.ascontiguousarray(dstl_w[k].astype(BFNP)),
            cnts=np.ascontiguousarray(cnts[k:k + 1]),
        ))
    return per_core, C_lo, C_hi, newid


# ------------------------------------------------------------- device build
def _build(C_lo, C_hi):
    T, lo_base, hi_base, groups = _layout(C_lo, C_hi)
    instrs = _instr_list(C_lo, C_hi)
    NI = len(instrs)
    CHMAX = max(g["nlo"] + g["nhi"] for g in groups)
    CLO_MAX = max(max(C_lo), 1)
    CHI_MAX = max(max(C_hi), 1)
    NG = len(groups)

    nc = bacc.Bacc("TRN2", target_bir_lowering=False, debug=False,
                   enable_asserts=False, num_devices=P,
                   dynamic_dma_scratch_size=65536)

    xpt_d = nc.dram_tensor("xpt", [IN_DIM, PADS], F32, kind="ExternalInput").ap()
    degp_d = nc.dram_tensor("degp", [BLK, NBLK], F32, kind="ExternalInput").ap()
    gidx_d = nc.dram_tensor("gidx", [BLK, 8 * T], I16, kind="ExternalInput").ap()
    dstl_d = nc.dram_tensor("dstl", [BLK, T], BF16, kind="ExternalInput").ap()
    cnts_d = nc.dram_tensor("cnts", [1, NI], I32, kind="ExternalInput").ap()
    w_d = [nc.dram_tensor(f"w{i}", [d, HID if i < 3 else OUT_DIM], F32,
                          kind="ExternalInput").ap()
           for i, d in enumerate([IN_DIM, HID, HID, HID])]
    bt_d = [nc.dram_tensor(f"bt{i}", [BLK, HID if i < 3 else OUT_DIM], F32,
                           kind="ExternalInput").ap()
            for i in range(4)]
    iota_d = nc.dram_tensor("iota", [BLK, BLK], BF16, kind="ExternalInput").ap()
    iden_d = nc.dram_tensor("iden", [BLK, BLK], BF16, kind="ExternalInput").ap()
    out_d = nc.dram_tensor("probs", [PADS, OUT_DIM], F32,
                           kind="ExternalOutput").ap()

    rg = [list(range(P))]

    with tile.TileContext(nc) as tc:
        with (
            tc.tile_pool(name="const", bufs=1) as cp,
            tc.tile_pool(name="xin", bufs=2) as xp_pool,
            tc.tile_pool(name="ht", bufs=3) as hp,
            tc.tile_pool(name="ztb", bufs=3) as zp,
            tc.tile_pool(name="oh", bufs=4) as ohp,
            tc.tile_pool(name="msg", bufs=3) as mp,
            tc.tile_pool(name="fin", bufs=4) as fp,
            tc.tile_pool(name="pstp", bufs=2, space="PSUM") as pstp,
            tc.tile_pool(name="psz", bufs=2, space="PSUM") as psz,
            tc.tile_pool(name="psacc", bufs=4, space="PSUM") as psacc,
            tc.tile_pool(name="dram", bufs=1, space="DRAM") as dp,
        ):
            # ---- constants
            w_sb, bt_sb = [], []
            for i in range(4):
                wt = cp.tile(list(w_d[i].shape), F32, tag=f"w{i}", name=f"w{i}")
                nc.sync.dma_start(wt[:], w_d[i])
                w_sb.append(wt)
                bt = cp.tile(list(bt_d[i].shape), F32, tag=f"bt{i}", name=f"bt{i}")
                nc.sync.dma_start(bt[:], bt_d[i])
                bt_sb.append(bt)
            wb_sb = {}
            for i in (1, 2, 3):
                wb = cp.tile(list(w_d[i].shape), BF16, tag=f"wb{i}", name=f"wb{i}")
                nc.vector.tensor_copy(wb[:], w_sb[i][:])
                wb_sb[i] = wb

            iota_sb = cp.tile([BLK, BLK], BF16, tag="iota")
            nc.sync.dma_start(iota_sb[:], iota_d)
            iden_sb = cp.tile([BLK, BLK], BF16, tag="iden")
            nc.sync.dma_start(iden_sb[:], iden_d)
            gidx_sb = cp.tile([BLK, 8 * T], I16, tag="gidx")
            nc.sync.dma_start(gidx_sb[:], gidx_d)
            dstl_sb = cp.tile([BLK, T], BF16, tag="dstl")
            nc.sync.dma_start(dstl_sb[:], dstl_d)
            cnts_sb = cp.tile([1, NI], I32, tag="cnts")
            nc.sync.dma_start(cnts_sb[:], cnts_d)
            creg = nc.gpsimd.alloc_register("gcnt") if REGCNT else None

            deg_sb = cp.tile([BLK, NBLK], F32, tag="deg")
            nc.sync.dma_start(deg_sb[:], degp_d)
            dinv_sb = cp.tile([BLK, NBLK], F32, tag="dinv")
            nc.vector.reciprocal(dinv_sb[:], deg_sb[:])
            nc.scalar.activation(dinv_sb[:], dinv_sb[:],
                                 mybir.ActivationFunctionType.Sqrt)

            zt_own = cp.tile([BLK, NBLK * HID], F32, tag="zt_own")
            h_sb = [cp.tile([BLK, NBLK * HID], BF16, tag=f"h{i}", name=f"h{i}")
                    for i in range(2)]

            agA = [dp.tile([ABLK * BLK, 2 * HID], BF16, tag=f"agA{l}",
                           name=f"agA{l}") for l in range(3)]
            agB = [dp.tile([BBLK * BLK, 2 * HID], BF16, tag=f"agB{l}",
                           name=f"agB{l}") for l in range(3)]
            tblA = [dp.tile([AROWS, 2 * HID], BF16, tag=f"tA{l}", name=f"tA{l}",
                            addr_space="Shared") for l in range(3)]
            tblB = [dp.tile([BROWS, 2 * HID], BF16, tag=f"tB{l}", name=f"tB{l}",
                            addr_space="Shared") for l in range(3)]

            # msg tiles are per-group; gathers for group g+1 are emitted
            # before the scatter of group g, so keep handles per group.
            msg_t = {}

            def ag_fire(l, half):
                src = agA[l] if half == 0 else agB[l]
                dst = tblA[l] if half == 0 else tblB[l]
                nc.gpsimd.collective_compute(
                    "AllGather", mybir.AluOpType.bypass, replica_groups=rg,
                    ins=[src.opt()], outs=[dst.opt()])

            def stage_table_row(b, z_ps, l_next, fire_ag=False):
                sl = slice(b * HID, (b + 1) * HID)
                nc.vector.tensor_scalar(zt_own[:, sl], z_ps[:],
                                        dinv_sb[:, b:b + 1], None,
                                        mybir.AluOpType.mult)
                ztb = zp.tile([BLK, 2 * HID], BF16, tag="ztb", name="ztb")
                nc.vector.tensor_copy(
                    ztb[:].rearrange("p (d e) -> p d e", e=HID),
                    zt_own[:, sl].unsqueeze(1).broadcast_to([BLK, 2, HID]))
                if b < ABLK:
                    nc.sync.dma_start(
                        agA[l_next][b * BLK:(b + 1) * BLK, :], ztb[:])
                else:
                    nc.sync.dma_start(
                        agB[l_next][(b - ABLK) * BLK:(b - ABLK + 1) * BLK, :],
                        ztb[:])
                if fire_ag and b == ABLK - 1:
                    ag_fire(l_next, 0)
                elif fire_ag and b == NBLK - 1:
                    ag_fire(l_next, 1)

            def transform_block(b, h_cur, l_next):
                tp_ps = pstp.tile([HID, BLK], BF16, tag="tp", name="tp")
                nc.tensor.transpose(tp_ps[:], h_cur[:, b * HID:(b + 1) * HID],
                                    iden_sb[:])
                hT = hp.tile([HID, BLK], BF16, tag="hT", name="hT")
                nc.vector.tensor_copy(hT[:], tp_ps[:])
                z_ps = psz.tile([BLK, HID], F32, tag="z", name="z_ps")
                nc.tensor.matmul(z_ps[:], hT[:], wb_sb[l_next][:],
                                 start=True, stop=True)
                stage_table_row(b, z_ps, l_next)

            def readout_block(h_cur, b):
                tp_ps = pstp.tile([HID, BLK], BF16, tag="tp", name="tp")
                nc.tensor.transpose(tp_ps[:], h_cur[:, b * HID:(b + 1) * HID],
                                    iden_sb[:])
                hT = hp.tile([HID, BLK], BF16, tag="hT", name="hT")
                nc.vector.tensor_copy(hT[:], tp_ps[:])
                o_ps = psz.tile([BLK, HID], F32, tag="z", name="o_ps")
                nc.tensor.matmul(o_ps[:, :OUT_DIM], hT[:], wb_sb[3][:],
                                 start=True, stop=True)
                logit = fp.tile([BLK, OUT_DIM], F32, tag="logit", name="logit")
                nc.vector.tensor_tensor(logit[:], o_ps[:, :OUT_DIM], bt_sb[3][:],
                                        mybir.AluOpType.add)
                nmx = fp.tile([BLK, 1], F32, tag="nmx", name="nmx")
                nc.vector.reduce_max(nmx[:], logit[:],
                                     axis=mybir.AxisListType.X, negate=True)
                ex = fp.tile([BLK, OUT_DIM], F32, tag="ex", name="ex")
                ssum = fp.tile([BLK, 1], F32, tag="ssum", name="ssum")
                nc.scalar.activation(ex[:], logit[:],
                                     mybir.ActivationFunctionType.Exp,
                                     bias=nmx[:], accum_out=ssum[:])
                rs = fp.tile([BLK, 1], F32, tag="rs", name="rs")
                nc.vector.reciprocal(rs[:], ssum[:])
                prob = fp.tile([BLK, OUT_DIM], F32, tag="prob", name="prob")
                nc.vector.tensor_scalar(prob[:], ex[:], rs[:], None,
                                        mybir.AluOpType.mult)
                nc.sync.dma_start(out_d[b * BLK:(b + 1) * BLK, :], prob[:])

            # per-(block,stream) instruction ids
            instr_ids = {}
            for i, (b, s, cbase, nch, soff) in enumerate(instrs):
                instr_ids.setdefault((b, s), []).append(i)

            def emit_gathers(l, gi, s):
                """gathers for group gi, stream s (0=lo from tblA,
                1=hi from tblB) of layer l"""
                g = groups[gi]
                if (l, gi) not in msg_t:
                    msg_t[(l, gi)] = mp.tile([BLK, CHMAX * 2 * HID], BF16,
                                             tag="msg", name=f"msg{l}_{gi}")
                    if REGCNT:
                        # rows skipped by short reg counts must never read
                        # NaN bit patterns into the scatter matmul
                        nc.vector.memzero(msg_t[(l, gi)][:])
                msg = msg_t[(l, gi)]
                table = (tblA[l] if s == 0 else tblB[l])
                for b in g["bs"]:
                    for i in instr_ids[(b, s)]:
                        _, _, cbase, nch, soff = instrs[i]
                        col = cbase - g["start"]
                        if REGCNT:
                            nc.gpsimd.reg_load(creg, cnts_sb[:, i:i + 1])
                            nreg = creg
                        else:
                            nreg = nch * BLK
                        nc.gpsimd.dma_gather(
                            msg[:, col * 2 * HID:(col + nch) * 2 * HID]
                                .rearrange("p (c e) -> p c e", e=2 * HID),
                            table[:, :],
                            gidx_sb[:, 8 * cbase:8 * (cbase + nch)],
                            num_idxs=nch * BLK,
                            num_idxs_reg=nreg,
                            elem_size=2 * HID)

            def emit_scatter(l, gi, h_nxt):
                g = groups[gi]
                msg = msg_t.pop((l, gi))
                for b in g["bs"]:
                    n_tot = C_lo[b] + C_hi[b]
                    agg_ps = psacc.tile([BLK, HID], F32, tag="acc",
                                        name="agg_ps")
                    k = 0
                    for (C, cmax, base) in ((C_lo[b], CLO_MAX, lo_base[b]),
                                            (C_hi[b], CHI_MAX, hi_base[b])):
                        oh = ohp.tile([BLK, cmax * BLK], BF16,
                                      tag=f"oh{cmax}", name="oh")
                        nc.vector.tensor_tensor(
                            oh[:, :C * BLK].rearrange("p (c e) -> p c e", e=BLK),
                            iota_sb[:].unsqueeze(1).broadcast_to([BLK, C, BLK]),
                            dstl_sb[:, base:base + C].unsqueeze(2)
                                .broadcast_to([BLK, C, BLK]),
                            mybir.AluOpType.is_equal)
                        for c in range(C):
                            mc = base - g["start"] + c
                            nc.tensor.matmul(
                                agg_ps[:],
                                oh[:, c * BLK:(c + 1) * BLK],
                                msg[:, mc * 2 * HID:mc * 2 * HID + HID],
                                start=(k == 0), stop=(k == n_tot - 1))
                            k += 1
                    sl = slice(b * HID, (b + 1) * HID)
                    tot = fp.tile([BLK, HID], F32, tag="tot", name="tot")
                    nc.vector.tensor_tensor(tot[:], agg_ps[:], zt_own[:, sl],
                                            mybir.AluOpType.add)
                    pre = fp.tile([BLK, HID], F32, tag="pre", name="pre")
                    nc.vector.scalar_tensor_tensor(
                        pre[:], tot[:], dinv_sb[:, b:b + 1], bt_sb[l][:],
                        mybir.AluOpType.mult, mybir.AluOpType.add)
                    nc.scalar.activation(h_nxt[:, sl], pre[:],
                                         mybir.ActivationFunctionType.Relu)
                    if l < 2:
                        transform_block(b, h_nxt, l + 1)
                    else:
                        readout_block(h_nxt, b)

            # ---------------- layer-0 transform from x
            for gi, g in enumerate(groups):
                bs = g["bs"]
                xg = xp_pool.tile([IN_DIM, G * BLK], F32, tag="xb", name="xb")
                nc.sync.dma_start(xg[:, :len(bs) * BLK],
                                  xpt_d[:, bs[0] * BLK:(bs[-1] + 1) * BLK])
                for j, b in enumerate(bs):
                    z_ps = psz.tile([BLK, HID], F32, tag="z", name="z_ps")
                    nc.tensor.matmul(z_ps[:], xg[:, j * BLK:(j + 1) * BLK],
                                     w_sb[0][:], start=True, stop=True)
                    stage_table_row(b, z_ps, 0, fire_ag=True)

            # ---------------- layers: staggered lo/hi gather emission
            for l in range(3):
                h_nxt = h_sb[l % 2]
                emit_gathers(l, 0, 0)
                emit_gathers(l, 1, 0)
                if l > 0:
                    # tblB[l] staged during layer l-1; waits are satisfied,
                    # collective flies while the lo gathers below proceed
                    ag_fire(l, 1)
                emit_gathers(l, 0, 1)
                emit_scatter(l, 0, h_nxt)
                for gi in range(2, NG):
                    emit_gathers(l, gi, 0)
                    emit_gathers(l, gi - 1, 1)
                    emit_scatter(l, gi - 1, h_nxt)
                    if l < 2 and gi == NG - 2:
                        # block-31 epilogue of this layer is done by the time
                        # Pool reaches here; AG-A(l+1) overlaps the tail
                        ag_fire(l + 1, 0)
                emit_gathers(l, NG - 1, 1)
                emit_scatter(l, NG - 1, h_nxt)

    nc.compile()
    return nc


# ------------------------------------------------------------- entry point
_CACHE = {}


def _get_program(key):
    if key not in _CACHE:
        _CACHE[key] = _build(*key)
    return _CACHE[key]


def _in_maps(per_core, W1, b1, W2, b2, W3, b3, Wr, br):
    ws = [np.asarray(w, np.float32) for w in (W1, W2, W3, Wr)]
    bts = [np.tile(np.asarray(b, np.float32).reshape(1, -1), (BLK, 1))
           for b in (b1, b2, b3, br)]
    iota = np.tile(np.arange(BLK, dtype=BFNP), (BLK, 1))
    iden = np.eye(BLK, dtype=BFNP)
    in_maps = []
    for k in range(P):
        m = dict(per_core[k])
        for i in range(4):
            m[f"w{i}"] = ws[i]
            m[f"bt{i}"] = bts[i]
        m["iota"] = iota
        m["iden"] = iden
        in_maps.append(m)
    return in_maps


def kernel(x, edge_index, W1, b1, W2, b2, W3, b3, Wr, br, trace=False):
    per_core, C_lo, C_hi, newid = _preprocess(x, edge_index)
    nc = _get_program((C_lo, C_hi))
    in_maps = _in_maps(per_core, W1, b1, W2, b2, W3, b3, Wr, br)
    res = run_bass_kernel_spmd(nc, in_maps, core_ids=list(range(P)),
                               trace=trace)
    allp = np.concatenate([res.results[k]["probs"] for k in range(P)], axis=0)
    out = allp[newid]
    kernel.last_results = res
    return out
